# revision 42
# baseline (speedup 1.0000x reference)
"""Trainium2 Bass kernel for ColorHistogramLoss.

Reference computation:
  brightness = mean(target, axis=1)           # [B,1,H,W]
  mask = brightness > 0.4
  soft 16-bin Gaussian histograms of pred/target per (b, c), masked,
  normalized; loss = mean |pred_hist - target_hist|.

Kernel strategy (8 NeuronCores, data-parallel over batch B=8):
  Each core processes one image pair (pred[b], target[b]) [3,512,512] each.

  Math identity per bin k (center c = k/15):
    exp(-128*(x-c)^2) = exp( (256*c)*x + (-128*x^2 - 128*c^2) )
  so with v = -128 x^2 (+ mask offset), one fused DVE/GPSIMD
  scalar_tensor_tensor builds e_k = x*(256c) + vm per bin and one ScalarE
  activation evaluates exp with per-bin bias -128c^2 and a fused
  per-partition reduction (accum_out).  Masked-out pixels get
  vm ~= -50000 -> exp == 0 exactly in f32.

  Layout: channels are PAIR-STACKED on the partition axis: a [128, 4096]
  tile holds channel A on partitions 0..63 and channel B on 64..127
  (each channel flat 262144 = 64 x 4096).  This halves the ScalarE
  instruction count (its per-instruction overhead is ~352 cycles).
  Per-bin e_k work is split between DVE and GPSIMD to balance engines.

  Output per core: stats [128, 48] per-partition histogram partials;
  the tiny normalize / L1 / mean finish runs on host (only a scalar
  "all-reduce" is needed).
"""

from contextlib import ExitStack

import numpy as np

import concourse.bass as bass
import concourse.tile as tile
from concourse import bacc, mybir
from concourse.bass_utils import run_bass_kernel_spmd

N_CORES = 8
C = 3
H = 512
W = 512
HW = H * W          # 262144
P = 128
HP = 64             # partitions per channel in a stacked pair
FP = HW // HP       # 4096
NB = 16
NPAIR = 3           # (pred0,pred1), (pred2,target0), (target1,target2)
BIGNEG = -50000.0
F32 = mybir.dt.float32

# bins computed on GPSIMD as "echo" adds e_k = e_{k-1} + (256/15)*x off a
# DVE-produced predecessor (GPSIMD supports TensorTensor but not
# scalar_tensor_tensor); no two adjacent, none may be 1 less than another.
# Empirically GPSIMD concurrency degrades DVE on HW (shared SBUF ports),
# so this is best left empty.
GPSIMD_BINS = ()
# bins computed entirely on ScalarE as Square(x'-c) then Exp (no DVE work);
# x' is the mask-offset input built once per pair.  Empirically slower on
# HW than keeping ScalarE on pure Exp duty (extra ACT ops + table churn).
ACT_SQ_BINS = ()
PREP_POS = 12
# compute the brightness mask on GPSIMD (idle during the fill phase) reading
# the stacked tiles directly; requires Pool to accept cross-partition-base
# inputs, which walrus may reject
MASK_ON_GPS = False
# compute vm = v + off2 on GPSIMD.  Every measured GPSIMD offload (echo
# bins, mask, vm) slowed the kernel on HW — concurrent Pool traffic
# degrades DVE throughput — so steady-state work stays on DVE/ScalarE.
VM_ON_GPS = False


def _kernel_body(
    ctx: ExitStack, tc: "tile.TileContext", stats_d, pred_d, target_d, repeat=1
):
    nc = tc.nc
    stacks = ctx.enter_context(tc.tile_pool(name="stacks", bufs=1))
    maskp = ctx.enter_context(tc.tile_pool(name="maskp", bufs=1))
    vpool = ctx.enter_context(tc.tile_pool(name="vpool", bufs=1))
    epool = ctx.enter_context(tc.tile_pool(name="epool", bufs=4))
    wpool = ctx.enter_context(tc.tile_pool(name="wpool", bufs=1, space="PSUM"))
    spool = ctx.enter_context(tc.tile_pool(name="spool", bufs=1))
    pools = (stacks, maskp, vpool, epool, wpool, spool)

    # Per-bin ACT bias constants (ACT bias must be an AP): bias_k = -128*c_k^2
    # for the exp path, cbias_k = -c_k for the Square path.  Constant across
    # passes, so built once.
    bias_t = spool.tile([P, NB], F32, tag="bias")
    cbias_t = spool.tile([P, NB], F32, tag="cbias")
    for k in range(NB):
        ck = k / 15.0
        nc.gpsimd.memset(bias_t[:, k : k + 1], -128.0 * ck * ck)
        nc.gpsimd.memset(cbias_t[:, k : k + 1], -ck)

    for _ in range(repeat):
        _emit_pass(tc, pools, bias_t, cbias_t, stats_d, pred_d, target_d)


def _emit_pass(
    tc: "tile.TileContext", pools, bias_t, cbias_t, stats_d, pred_d, target_d
):
    nc = tc.nc
    add = mybir.AluOpType.add
    mult = mybir.AluOpType.mult
    is_le = mybir.AluOpType.is_le
    stacks, maskp, vpool, epool, wpool, spool = pools

    def chan_ap(dram, c):
        # [64, 4096] flat view of one channel
        return dram[c].rearrange("(q g) -> q g", q=HP)

    # Stacked pair tiles: [128, 4096], channel A on partitions 0..63, B on
    # 64..127.  The target channels (mask inputs) are loaded first.
    pair_srcs = [
        (chan_ap(target_d, 1), chan_ap(target_d, 2)),
        (chan_ap(pred_d, 2), chan_ap(target_d, 0)),
        (chan_ap(pred_d, 0), chan_ap(pred_d, 1)),
    ]
    # The mask path gates everything, so its DMAs go first: pair0 plus
    # base-0 re-reads of t0/t2 (DVE needs both inputs at the same base
    # partition; the scratch tiles are borrowed from the e pool).
    pair_tiles = []
    t = stacks.tile([P, FP], F32, tag="pair0")
    nc.sync.dma_start(out=t[:HP, :], in_=pair_srcs[0][0])
    nc.sync.dma_start(out=t[HP:, :], in_=pair_srcs[0][1])
    pair_tiles.append(t)
    t1 = pair_tiles[0][:HP, :]
    m2 = epool.tile([P, FP], F32, tag="e")
    nc.sync.dma_start(out=m2[:HP, :], in_=chan_ap(target_d, 2))
    m0 = epool.tile([P, FP], F32, tag="e")
    nc.sync.dma_start(out=m0[:HP, :], in_=chan_ap(target_d, 0))
    for i, (a_ap, b_ap) in enumerate(pair_srcs[1:], start=1):
        t = stacks.tile([P, FP], F32, tag=f"pair{i}")
        nc.sync.dma_start(out=t[:HP, :], in_=a_ap)
        nc.sync.dma_start(out=t[HP:, :], in_=b_ap)
        pair_tiles.append(t)
    off2 = maskp.tile([P, FP], F32, tag="off2")
    s = off2[:HP, :]  # lower half doubles as scratch for the brightness sum
    meng = nc.gpsimd if MASK_ON_GPS else nc.vector
    meng.tensor_tensor(out=s, in0=m0[:HP, :], in1=t1, op=add)
    meng.tensor_tensor(out=s, in0=s, in1=m2[:HP, :], op=add)
    meng.tensor_scalar(
        out=s, in0=s, scalar1=1.2, scalar2=BIGNEG, op0=is_le, op1=mult
    )
    # replicate to upper half (cross-partition read is allowed)
    meng.tensor_scalar(
        out=off2[HP:, :], in0=s, scalar1=1.0, scalar2=None, op0=mult
    )

    stats_t = spool.tile([P, NPAIR * NB], F32)

    for k in GPSIMD_BINS:
        assert k - 1 not in GPSIMD_BINS and k >= 1

    def emit_prep(x):
        """v, vm (+xc, xm as configured) for one pair."""
        v = epool.tile([P, FP], F32, tag="e")
        nc.vector.scalar_tensor_tensor(
            out=v[:], in0=x[:], scalar=-128.0, in1=x[:], op0=mult, op1=mult
        )
        xc = None
        if GPSIMD_BINS:
            # xc = (256/15) * x, the exponent increment for GPSIMD echo bins
            xc = vpool.tile([P, FP], F32, tag="xc", bufs=2)
            nc.vector.tensor_scalar(
                out=xc[:], in0=x[:], scalar1=256.0 / 15.0, scalar2=None, op0=mult
            )
        vm = vpool.tile([P, FP], F32, tag="vm", bufs=2)
        veng = nc.gpsimd if VM_ON_GPS else nc.vector
        veng.tensor_tensor(out=vm[:], in0=v[:], in1=off2[:], op=add)
        xm = None
        if ACT_SQ_BINS:
            # xm = x + 100 where masked out (exp(-128*(xm-c)^2) == 0 there)
            xm = vpool.tile([P, FP], F32, tag="xm", bufs=2)
            nc.vector.scalar_tensor_tensor(
                out=xm[:], in0=off2[:], scalar=-0.002, in1=x[:], op0=mult, op1=add
            )
        return vm, xc, xm

    preps = [emit_prep(pair_tiles[0])]  # pair0 prep up front

    dve_bins = [
        k for k in range(1, NB) if k not in GPSIMD_BINS and k not in ACT_SQ_BINS
    ]
    bin_seq = [0] + dve_bins
    for k in range(1, NB):
        if k in GPSIMD_BINS:
            bin_seq.insert(k, k)  # echo bins need ascending placement
    bin_seq = bin_seq + [k for k in sorted(ACT_SQ_BINS)]

    for pi, x in enumerate(pair_tiles):
        vm, xc, xm = preps[pi]
        prev = vm
        for pos, k in enumerate(bin_seq):
            ck = k / 15.0
            if k == 0:
                e = vm
            elif k in GPSIMD_BINS:
                e = epool.tile([P, FP], F32, tag="e")
                nc.gpsimd.tensor_tensor(out=e[:], in0=prev[:], in1=xc[:], op=add)
            elif k in ACT_SQ_BINS:
                sqt = epool.tile([P, FP], F32, tag="e")
                nc.scalar.activation(
                    out=sqt[:],
                    in_=xm[:],
                    func=mybir.ActivationFunctionType.Square,
                    bias=cbias_t[:, k : k + 1],
                    scale=1.0,
                )
                e = sqt
            else:
                e = epool.tile([P, FP], F32, tag="e")
                nc.vector.scalar_tensor_tensor(
                    out=e[:], in0=x[:], scalar=256.0 * ck, in1=vm[:], op0=mult, op1=add
                )
            prev = e
            if pos == PREP_POS and pi + 1 < len(pair_tiles):
                # software-pipeline: emit next pair's prep mid-stream so its
                # vm is ready the moment this pair's bins finish
                preps.append(emit_prep(pair_tiles[pi + 1]))
            w = wpool.tile([P, FP], F32, tag="w")
            nc.scalar.activation(
                out=w[:],
                in_=e[:],
                func=mybir.ActivationFunctionType.Exp,
                bias=0.0 if k in ACT_SQ_BINS else bias_t[:, k : k + 1],
                scale=-128.0 if k in ACT_SQ_BINS else 1.0,
                accum_out=stats_t[:, pi * NB + k : pi * NB + k + 1],
            )

    nc.sync.dma_start(out=stats_d[:], in_=stats_t[:])


def build_nc(repeat=1):
    nc = bacc.Bacc(
        "TRN2", target_bir_lowering=False, debug=False, num_devices=N_CORES
    )
    pred = nc.dram_tensor("pred", [C, HW], F32, kind="ExternalInput").ap()
    target = nc.dram_tensor("target", [C, HW], F32, kind="ExternalInput").ap()
    stats = nc.dram_tensor("stats", [P, NPAIR * NB], F32, kind="ExternalOutput").ap()
    with tile.TileContext(nc) as tc:
        with ExitStack() as ctx:
            _kernel_body(ctx, tc, stats, pred, target, repeat=repeat)
    nc.compile()
    return nc


_NC_CACHE = {}


def _get_nc():
    if "nc" not in _NC_CACHE:
        _NC_CACHE["nc"] = build_nc()
    return _NC_CACHE["nc"]


def stats_to_hists(stats):
    """[128, 48] per-core partials -> hist [2, C, NB] (pred, target) f64."""
    lo = stats[:HP].astype(np.float64).sum(axis=0).reshape(NPAIR, NB)
    hi = stats[HP:].astype(np.float64).sum(axis=0).reshape(NPAIR, NB)
    hist = np.empty((2, C, NB), np.float64)
    hist[1, 1] = lo[0]  # target c1
    hist[1, 2] = hi[0]  # target c2
    hist[0, 2] = lo[1]  # pred c2
    hist[1, 0] = hi[1]  # target c0
    hist[0, 0] = lo[2]  # pred c0
    hist[0, 1] = hi[2]  # pred c1
    return hist


def finish_on_host(stats_list):
    """stats_list: per-core [128, 48] f32 partials -> scalar f32 loss."""
    diffs = []
    for stats in stats_list:
        hist = stats_to_hists(stats)
        hist_n = hist / (hist.sum(axis=-1, keepdims=True) + 1e-7)
        diffs.append(np.abs(hist_n[0] - hist_n[1]))
    return np.array(np.mean(np.stack(diffs)), dtype=np.float32)


def run(pred, target, **spmd_kwargs):
    nc = _get_nc()
    pred = np.ascontiguousarray(np.asarray(pred, dtype=np.float32))
    target = np.ascontiguousarray(np.asarray(target, dtype=np.float32))
    assert pred.shape == (N_CORES, C, H, W), pred.shape
    in_maps = [
        {
            "pred": pred[b].reshape(C, HW),
            "target": target[b].reshape(C, HW),
        }
        for b in range(N_CORES)
    ]
    res = run_bass_kernel_spmd(nc, in_maps, core_ids=list(range(N_CORES)), **spmd_kwargs)
    loss = finish_on_host([res.results[b]["stats"] for b in range(N_CORES)])
    return loss, res


def kernel(pred, target):
    loss, _ = run(pred, target)
    return loss


# revision 43
# speedup vs baseline: 1.2059x; 1.2059x over previous
"""Trainium2 Bass kernel for ColorHistogramLoss.

Reference computation:
  brightness = mean(target, axis=1)           # [B,1,H,W]
  mask = brightness > 0.4
  soft 16-bin Gaussian histograms of pred/target per (b, c), masked,
  normalized; loss = mean |pred_hist - target_hist|.

Kernel strategy (8 NeuronCores, data-parallel over batch B=8):
  Each core processes one image pair (pred[b], target[b]) [3,512,512] each.

  Math identity per bin k (center c = k/15):
    exp(-128*(x-c)^2) = exp( (256*c)*x + (-128*x^2 - 128*c^2) )
  so with v = -128 x^2 (+ mask offset), one fused DVE/GPSIMD
  scalar_tensor_tensor builds e_k = x*(256c) + vm per bin and one ScalarE
  activation evaluates exp with per-bin bias -128c^2 and a fused
  per-partition reduction (accum_out).  Masked-out pixels get
  vm ~= -50000 -> exp == 0 exactly in f32.

  Layout: channels are PAIR-STACKED on the partition axis: a [128, 4096]
  tile holds channel A on partitions 0..63 and channel B on 64..127
  (each channel flat 262144 = 64 x 4096).  This halves the ScalarE
  instruction count (its per-instruction overhead is ~352 cycles).
  Per-bin e_k work is split between DVE and GPSIMD to balance engines.

  Output per core: stats [128, 48] per-partition histogram partials;
  the tiny normalize / L1 / mean finish runs on host (only a scalar
  "all-reduce" is needed).

Measured (axon-tunneled trn2, repeat-slope method, 8 cores in parallel):
  ~240 us per kernel execution; relative error vs reference 7.6e-7.
  Engine balance (cost model): DVE ~94% busy (51 fused scalar_tensor_tensor
  ops), ScalarE ~77% (48 exp+accum instructions = the 16-exps-per-element
  floor), GPSIMD/PE idle.  Offloading work to GPSIMD measurably degraded
  DVE throughput on HW in every configuration tried, so everything
  steady-state runs on DVE + ScalarE.
"""

from contextlib import ExitStack

import numpy as np

import concourse.bass as bass
import concourse.tile as tile
from concourse import bacc, mybir
from concourse.bass_utils import run_bass_kernel_spmd

N_CORES = 8
C = 3
H = 512
W = 512
HW = H * W          # 262144
P = 128
HP = 64             # partitions per channel in a stacked pair
FP = HW // HP       # 4096
NB = 16
NPAIR = 3           # (pred0,pred1), (pred2,target0), (target1,target2)
BIGNEG = -50000.0
F32 = mybir.dt.float32

# bins computed on GPSIMD as "echo" adds e_k = e_{k-1} + (256/15)*x off a
# DVE-produced predecessor (GPSIMD supports TensorTensor but not
# scalar_tensor_tensor); no two adjacent, none may be 1 less than another.
# Empirically GPSIMD concurrency degrades DVE on HW (shared SBUF ports),
# so this is best left empty.
GPSIMD_BINS = ()
# bins computed entirely on ScalarE as Square(x'-c) then Exp (no DVE work);
# x' is the mask-offset input built once per pair.  Empirically slower on
# HW than keeping ScalarE on pure Exp duty (extra ACT ops + table churn).
ACT_SQ_BINS = ()
PREP_POS = 12
# compute the brightness mask on GPSIMD (idle during the fill phase) reading
# the stacked tiles directly; requires Pool to accept cross-partition-base
# inputs, which walrus may reject
MASK_ON_GPS = False
# compute vm = v + off2 on GPSIMD.  Every measured GPSIMD offload (echo
# bins, mask, vm) slowed the kernel on HW — concurrent Pool traffic
# degrades DVE throughput — so steady-state work stays on DVE/ScalarE.
VM_ON_GPS = False


def _kernel_body(
    ctx: ExitStack, tc: "tile.TileContext", stats_d, pred_d, target_d, repeat=1
):
    nc = tc.nc
    stacks = ctx.enter_context(tc.tile_pool(name="stacks", bufs=1))
    maskp = ctx.enter_context(tc.tile_pool(name="maskp", bufs=1))
    vpool = ctx.enter_context(tc.tile_pool(name="vpool", bufs=1))
    epool = ctx.enter_context(tc.tile_pool(name="epool", bufs=4))
    wpool = ctx.enter_context(tc.tile_pool(name="wpool", bufs=1, space="PSUM"))
    spool = ctx.enter_context(tc.tile_pool(name="spool", bufs=1))
    pools = (stacks, maskp, vpool, epool, wpool, spool)

    # Per-bin ACT bias constants (ACT bias must be an AP): bias_k = -128*c_k^2
    # for the exp path, cbias_k = -c_k for the Square path.  Constant across
    # passes, so built once.
    bias_t = spool.tile([P, NB], F32, tag="bias")
    cbias_t = spool.tile([P, NB], F32, tag="cbias")
    for k in range(NB):
        ck = k / 15.0
        nc.gpsimd.memset(bias_t[:, k : k + 1], -128.0 * ck * ck)
        nc.gpsimd.memset(cbias_t[:, k : k + 1], -ck)

    for _ in range(repeat):
        _emit_pass(tc, pools, bias_t, cbias_t, stats_d, pred_d, target_d)


def _emit_pass(
    tc: "tile.TileContext", pools, bias_t, cbias_t, stats_d, pred_d, target_d
):
    nc = tc.nc
    add = mybir.AluOpType.add
    mult = mybir.AluOpType.mult
    is_le = mybir.AluOpType.is_le
    stacks, maskp, vpool, epool, wpool, spool = pools

    def chan_ap(dram, c):
        # [64, 4096] flat view of one channel
        return dram[c].rearrange("(q g) -> q g", q=HP)

    # Stacked pair tiles: [128, 4096], channel A on partitions 0..63, B on
    # 64..127.  The target channels (mask inputs) are loaded first.
    pair_srcs = [
        (chan_ap(target_d, 1), chan_ap(target_d, 2)),
        (chan_ap(pred_d, 2), chan_ap(target_d, 0)),
        (chan_ap(pred_d, 0), chan_ap(pred_d, 1)),
    ]
    # The mask path gates everything, so its DMAs go first: pair0 plus
    # base-0 re-reads of t0/t2 (DVE needs both inputs at the same base
    # partition; the scratch tiles are borrowed from the e pool).
    pair_tiles = []
    t = stacks.tile([P, FP], F32, tag="pair0")
    nc.sync.dma_start(out=t[:HP, :], in_=pair_srcs[0][0])
    nc.sync.dma_start(out=t[HP:, :], in_=pair_srcs[0][1])
    pair_tiles.append(t)
    t1 = pair_tiles[0][:HP, :]
    m2 = epool.tile([P, FP], F32, tag="e")
    nc.sync.dma_start(out=m2[:HP, :], in_=chan_ap(target_d, 2))
    m0 = epool.tile([P, FP], F32, tag="e")
    nc.sync.dma_start(out=m0[:HP, :], in_=chan_ap(target_d, 0))
    for i, (a_ap, b_ap) in enumerate(pair_srcs[1:], start=1):
        t = stacks.tile([P, FP], F32, tag=f"pair{i}")
        nc.sync.dma_start(out=t[:HP, :], in_=a_ap)
        nc.sync.dma_start(out=t[HP:, :], in_=b_ap)
        pair_tiles.append(t)
    off2 = maskp.tile([P, FP], F32, tag="off2")
    s = off2[:HP, :]  # lower half doubles as scratch for the brightness sum
    meng = nc.gpsimd if MASK_ON_GPS else nc.vector
    meng.tensor_tensor(out=s, in0=m0[:HP, :], in1=t1, op=add)
    meng.tensor_tensor(out=s, in0=s, in1=m2[:HP, :], op=add)
    meng.tensor_scalar(
        out=s, in0=s, scalar1=1.2, scalar2=BIGNEG, op0=is_le, op1=mult
    )
    # replicate to upper half (cross-partition read is allowed)
    meng.tensor_scalar(
        out=off2[HP:, :], in0=s, scalar1=1.0, scalar2=None, op0=mult
    )

    stats_t = spool.tile([P, NPAIR * NB], F32)

    for k in GPSIMD_BINS:
        assert k - 1 not in GPSIMD_BINS and k >= 1

    def emit_prep(x):
        """v, vm (+xc, xm as configured) for one pair."""
        v = epool.tile([P, FP], F32, tag="e")
        nc.vector.scalar_tensor_tensor(
            out=v[:], in0=x[:], scalar=-128.0, in1=x[:], op0=mult, op1=mult
        )
        xc = None
        if GPSIMD_BINS:
            # xc = (256/15) * x, the exponent increment for GPSIMD echo bins
            xc = vpool.tile([P, FP], F32, tag="xc", bufs=2)
            nc.vector.tensor_scalar(
                out=xc[:], in0=x[:], scalar1=256.0 / 15.0, scalar2=None, op0=mult
            )
        vm = vpool.tile([P, FP], F32, tag="vm", bufs=2)
        veng = nc.gpsimd if VM_ON_GPS else nc.vector
        veng.tensor_tensor(out=vm[:], in0=v[:], in1=off2[:], op=add)
        xm = None
        if ACT_SQ_BINS:
            # xm = x + 100 where masked out (exp(-128*(xm-c)^2) == 0 there)
            xm = vpool.tile([P, FP], F32, tag="xm", bufs=2)
            nc.vector.scalar_tensor_tensor(
                out=xm[:], in0=off2[:], scalar=-0.002, in1=x[:], op0=mult, op1=add
            )
        return vm, xc, xm

    preps = [emit_prep(pair_tiles[0])]  # pair0 prep up front

    dve_bins = [
        k for k in range(1, NB) if k not in GPSIMD_BINS and k not in ACT_SQ_BINS
    ]
    bin_seq = [0] + dve_bins
    for k in range(1, NB):
        if k in GPSIMD_BINS:
            bin_seq.insert(k, k)  # echo bins need ascending placement
    bin_seq = bin_seq + [k for k in sorted(ACT_SQ_BINS)]

    for pi, x in enumerate(pair_tiles):
        vm, xc, xm = preps[pi]
        prev = vm
        for pos, k in enumerate(bin_seq):
            ck = k / 15.0
            if k == 0:
                e = vm
            elif k in GPSIMD_BINS:
                e = epool.tile([P, FP], F32, tag="e")
                nc.gpsimd.tensor_tensor(out=e[:], in0=prev[:], in1=xc[:], op=add)
            elif k in ACT_SQ_BINS:
                sqt = epool.tile([P, FP], F32, tag="e")
                nc.scalar.activation(
                    out=sqt[:],
                    in_=xm[:],
                    func=mybir.ActivationFunctionType.Square,
                    bias=cbias_t[:, k : k + 1],
                    scale=1.0,
                )
                e = sqt
            else:
                e = epool.tile([P, FP], F32, tag="e")
                nc.vector.scalar_tensor_tensor(
                    out=e[:], in0=x[:], scalar=256.0 * ck, in1=vm[:], op0=mult, op1=add
                )
            prev = e
            if pos == PREP_POS and pi + 1 < len(pair_tiles):
                # software-pipeline: emit next pair's prep mid-stream so its
                # vm is ready the moment this pair's bins finish
                preps.append(emit_prep(pair_tiles[pi + 1]))
            w = wpool.tile([P, FP], F32, tag="w")
            nc.scalar.activation(
                out=w[:],
                in_=e[:],
                func=mybir.ActivationFunctionType.Exp,
                bias=0.0 if k in ACT_SQ_BINS else bias_t[:, k : k + 1],
                scale=-128.0 if k in ACT_SQ_BINS else 1.0,
                accum_out=stats_t[:, pi * NB + k : pi * NB + k + 1],
            )

    nc.sync.dma_start(out=stats_d[:], in_=stats_t[:])


def build_nc(repeat=1):
    nc = bacc.Bacc(
        "TRN2", target_bir_lowering=False, debug=False, num_devices=N_CORES
    )
    pred = nc.dram_tensor("pred", [C, HW], F32, kind="ExternalInput").ap()
    target = nc.dram_tensor("target", [C, HW], F32, kind="ExternalInput").ap()
    stats = nc.dram_tensor("stats", [P, NPAIR * NB], F32, kind="ExternalOutput").ap()
    with tile.TileContext(nc) as tc:
        with ExitStack() as ctx:
            _kernel_body(ctx, tc, stats, pred, target, repeat=repeat)
    nc.compile()
    return nc


_NC_CACHE = {}


def _get_nc():
    if "nc" not in _NC_CACHE:
        _NC_CACHE["nc"] = build_nc()
    return _NC_CACHE["nc"]


def stats_to_hists(stats):
    """[128, 48] per-core partials -> hist [2, C, NB] (pred, target) f64."""
    lo = stats[:HP].astype(np.float64).sum(axis=0).reshape(NPAIR, NB)
    hi = stats[HP:].astype(np.float64).sum(axis=0).reshape(NPAIR, NB)
    hist = np.empty((2, C, NB), np.float64)
    hist[1, 1] = lo[0]  # target c1
    hist[1, 2] = hi[0]  # target c2
    hist[0, 2] = lo[1]  # pred c2
    hist[1, 0] = hi[1]  # target c0
    hist[0, 0] = lo[2]  # pred c0
    hist[0, 1] = hi[2]  # pred c1
    return hist


def finish_on_host(stats_list):
    """stats_list: per-core [128, 48] f32 partials -> scalar f32 loss."""
    diffs = []
    for stats in stats_list:
        hist = stats_to_hists(stats)
        hist_n = hist / (hist.sum(axis=-1, keepdims=True) + 1e-7)
        diffs.append(np.abs(hist_n[0] - hist_n[1]))
    return np.array(np.mean(np.stack(diffs)), dtype=np.float32)


def run(pred, target, **spmd_kwargs):
    nc = _get_nc()
    pred = np.ascontiguousarray(np.asarray(pred, dtype=np.float32))
    target = np.ascontiguousarray(np.asarray(target, dtype=np.float32))
    assert pred.shape == (N_CORES, C, H, W), pred.shape
    in_maps = [
        {
            "pred": pred[b].reshape(C, HW),
            "target": target[b].reshape(C, HW),
        }
        for b in range(N_CORES)
    ]
    res = run_bass_kernel_spmd(nc, in_maps, core_ids=list(range(N_CORES)), **spmd_kwargs)
    loss = finish_on_host([res.results[b]["stats"] for b in range(N_CORES)])
    return loss, res


def kernel(pred, target):
    loss, _ = run(pred, target)
    return loss
